# revision 4
# baseline (speedup 1.0000x reference)
"""Trainium2 Bass kernel for nn_LDS_LR: low-rank LDS + AR low-rank correction.

Math (per batch b):
    Bu   = X @ B1 @ B2                      # [T, N] rank-64 input projection
    h_t  = A * h_{t-1} + Bu_t               # diagonal recurrence, h_{-1} = h0
    lds  = H @ C1 @ C2                      # [T, O] rank-64 output projection
    proj = einsum('ti,rik->trk', X, M1)     # [T, R, KX]
    ar_t = sum_k M2[:,:,k] @ proj[t-k,:,k]  # AR with KX=5 taps
    Y    = lds + ar

Sharding: 8 cores = 4 batches x 2 sequence halves (1024 steps each).
Chunk-1 cores recompute the carry h_1023 on-device from the previous
half's inputs (rank-64 projection + a DVE carry scan), so the program is
fully uniform SPMD with no cross-core communication.

Device layout is transposed throughout: [feature, time]. Weights are
fused host-side:
    W1 [1024, 384] = [B1 | M1k0.T | ... | M1k4.T]   (input proj, one matmul)
    W2 [384, 1024] = [C2 ; M2k0.T ; ... ; M2k4.T]   (output proj, one matmul)
The AR delay taps become column shifts in the access patterns of the
final matmul's moving operand (P_ext keeps 4 leading boundary columns),
so no shift copies are ever materialized.
"""

import contextlib
import ctypes
import os
import sys
import types

import numpy as np
from contextlib import ExitStack

import concourse.bass as bass
import concourse.tile as tile
from concourse import bacc, mybir
from concourse.bass_utils import run_bass_kernel_spmd


def _install_ntff_hook():
    """Provide antenv.axon_hooks.get_axon_ntff_profile_hook if the image
    lacks it, driving NTFF capture via the libaxon_pjrt C ABI directly."""
    try:
        from antenv.axon_hooks import get_axon_ntff_profile_hook  # noqa: F401
        return
    except ImportError:
        pass
    so_path = "/opt/axon/libaxon_pjrt.so"
    hook = None
    if os.path.exists(so_path):
        lib = ctypes.CDLL(so_path)
        if hasattr(lib, "axon_start_nrt_profile"):
            lib.axon_start_nrt_profile.argtypes = [
                ctypes.POINTER(ctypes.c_int64), ctypes.c_size_t]
            lib.axon_start_nrt_profile.restype = ctypes.c_int64
            lib.axon_stop_nrt_profile.argtypes = [ctypes.c_char_p]
            lib.axon_stop_nrt_profile.restype = ctypes.c_int64

            @contextlib.contextmanager
            def _hook(output_dir, device_ids):
                import jax
                jax.devices()
                if device_ids:
                    ids = (ctypes.c_int64 * len(device_ids))(*device_ids)
                    rc = lib.axon_start_nrt_profile(ids, len(device_ids))
                else:
                    rc = lib.axon_start_nrt_profile(None, 0)
                if rc != 0:
                    raise RuntimeError(f"axon_start_nrt_profile rc={rc}")
                try:
                    yield
                finally:
                    n = lib.axon_stop_nrt_profile(str(output_dir).encode())
                    print(f"ntff profile: {n} file(s) -> {output_dir}",
                          file=sys.stderr)

            hook = _hook
    mod = types.ModuleType("antenv.axon_hooks")
    mod.get_axon_ntff_profile_hook = lambda: hook
    mod.set_axon_ntff_profile_hook = lambda h: None
    sys.modules["antenv.axon_hooks"] = mod


_install_ntff_hook()

DT = mybir.dt.float32
F32 = np.float32

B, T, D = 4, 2048, 1024
NST, R, KX, OUT = 1024, 64, 5, 1024
TC = 1024          # per-core chunk length
TBL = 512          # time block (one PSUM bank at fp32)
NT = TC // TBL     # 2 time blocks

_CACHED_NC = None
LAST_RESULT = None  # BassKernelResults of the most recent run (for test.py)

MULT = mybir.AluOpType.mult
ADD = mybir.AluOpType.add


def _emit(ctx, tc, io):
    nc = tc.nc
    xo, xp, w1, b2, c1, w2, av, iv, ioff, yt = io

    wp = ctx.enter_context(tc.tile_pool(name="wp", bufs=1))
    xpool = ctx.enter_context(tc.tile_pool(name="xpool", bufs=1))
    hp = ctx.enter_context(tc.tile_pool(name="hp", bufs=1))
    pp = ctx.enter_context(tc.tile_pool(name="pp", bufs=1))
    sc = ctx.enter_context(tc.tile_pool(name="sc", bufs=2))
    yp = ctx.enter_context(tc.tile_pool(name="yp", bufs=3))
    psA = ctx.enter_context(tc.tile_pool(name="psA", bufs=4, space="PSUM"))
    psB = ctx.enter_context(tc.tile_pool(name="psB", bufs=2, space="PSUM"))
    psT = ctx.enter_context(tc.tile_pool(name="psT", bufs=2, space="PSUM"))

    # ---- param / input loads -------------------------------------------------
    w1sb = []
    for k in range(8):
        t = wp.tile([128, 384], DT, tag=f"w1_{k}", name=f"w1_{k}")
        nc.sync.dma_start(t[:], w1[k * 128:(k + 1) * 128, :])
        w1sb.append(t)
    b2sb = wp.tile([64, 1024], DT, tag="b2", name="b2sb")
    nc.sync.dma_start(b2sb[:], b2[:])
    c1sb = []
    for k in range(8):
        t = wp.tile([128, 64], DT, tag=f"c1_{k}", name=f"c1_{k}")
        nc.sync.dma_start(t[:], c1[k * 128:(k + 1) * 128, :])
        c1sb.append(t)
    w2sb = []
    for m in range(3):
        t = wp.tile([128, 1024], DT, tag=f"w2_{m}", name=f"w2_{m}")
        nc.sync.dma_start(t[:], w2[m * 128:(m + 1) * 128, :])
        w2sb.append(t)
    avsb, ivsb, iosb = [], [], []
    for n in range(8):
        a = wp.tile([128, 1], DT, tag=f"av{n}", name=f"av{n}")
        nc.sync.dma_start(a[:], av[n * 128:(n + 1) * 128, :])
        avsb.append(a)
        v = wp.tile([128, 1], DT, tag=f"iv{n}", name=f"iv{n}")
        nc.sync.dma_start(v[:], iv[n * 128:(n + 1) * 128, :])
        ivsb.append(v)
        o = wp.tile([128, 1], DT, tag=f"io{n}", name=f"io{n}")
        nc.sync.dma_start(o[:], ioff[n * 128:(n + 1) * 128, :])
        iosb.append(o)
    xosb, xpsb = [], []
    for k in range(8):
        t = xpool.tile([128, 1024], DT, tag=f"xo{k}", name=f"xo{k}")
        nc.sync.dma_start(t[:], xo[k * 128:(k + 1) * 128, :])
        xosb.append(t)
        t = xpool.tile([128, 1024], DT, tag=f"xp{k}", name=f"xp{k}")
        nc.sync.dma_start(t[:], xp[k * 128:(k + 1) * 128, :])
        xpsb.append(t)

    # A broadcast along time: ab[n] = A[n-tile] * ones  (ScalarE, keeps DVE free)
    ones = wp.tile([128, 1024], DT, tag="ones", name="ones")
    nc.vector.memset(ones[:], 1.0)
    absb = []
    for n in range(8):
        ab = wp.tile([128, 1024], DT, tag=f"ab{n}", name=f"ab{n}")
        nc.scalar.mul(ab[:], ones[:], avsb[n][:])
        absb.append(ab)

    # ---- G_prev = (X_prev @ B1)^T : [64, 1024] ------------------------------
    gsb = wp.tile([64, 1024], DT, tag="gprev", name="gsb")
    for t in range(NT):
        g_ps = psB.tile([64, TBL], DT, tag="sm", name=f"g_ps{t}")
        for k in range(8):
            nc.tensor.matmul(g_ps[:], w1sb[k][:, 0:64],
                             xpsb[k][:, t * TBL:(t + 1) * TBL],
                             start=(k == 0), stop=(k == 7))
        nc.scalar.copy(gsb[:, t * TBL:(t + 1) * TBL], g_ps[:])

    # ---- carry: Bu_prev per n-tile + DVE carry scan -> initial_own ----------
    iown = []
    for n in range(8):
        prev_col = None
        for t in range(NT):
            bu_ps = psA.tile([128, TBL], DT, tag="mm", name=f"bup{n}_{t}")
            nc.tensor.matmul(bu_ps[:], b2sb[:, n * 128:(n + 1) * 128],
                             gsb[0:64, t * TBL:(t + 1) * TBL],
                             start=True, stop=True)
            cs = sc.tile([128, TBL], DT, tag="cscan", name=f"cs{n}_{t}")
            init = ivsb[n][:] if t == 0 else prev_col
            nc.vector.tensor_tensor_scan(
                cs[:], absb[n][:, t * TBL:(t + 1) * TBL], bu_ps[:], init,
                MULT, ADD)
            prev_col = cs[:, TBL - 1:TBL]
        it = wp.tile([128, 1], DT, tag=f"iown{n}", name=f"iown{n}")
        nc.vector.tensor_scalar_add(it[:], prev_col, iosb[n][:])
        iown.append(it)

    # ---- P_ext = (X_own @ W1)^T, AR taps pre-shifted at write time ----------
    # pext[j] rows 0:64 / 64:128 hold taps (2j-1, 2j) for j>0; tap k's value
    # for time s is stored at column 4+s+k, so the final Y matmul reads all
    # 128 rows at ONE column range [4+t*TBL, ...) — full K=128, base 0, no
    # row-group pairing (which hangs fp32 HW).  j0 = [Bu1 | tap0], shift 0.
    PW = 4 + TC + 4
    pext = [pp.tile([128, PW], DT, tag=f"pext{j}", name=f"pext{j}")
            for j in range(3)]

    def emit_p(j):
        for t in range(NT):
            p_ps = psA.tile([128, TBL], DT, tag="mm", name=f"p_ps{j}_{t}")
            for k in range(8):
                nc.tensor.matmul(p_ps[:], w1sb[k][:, j * 128:(j + 1) * 128],
                                 xosb[k][:, t * TBL:(t + 1) * TBL],
                                 start=(k == 0), stop=(k == 7))
            if j == 0:
                nc.scalar.copy(pext[0][:, 4 + t * TBL:4 + (t + 1) * TBL],
                               p_ps[:])
            else:
                ka, kb = 2 * j - 1, 2 * j
                nc.scalar.copy(
                    pext[j][0:64, 4 + ka + t * TBL:4 + ka + (t + 1) * TBL],
                    p_ps[0:64, :])
                nc.scalar.copy(
                    pext[j][64:128, 4 + kb + t * TBL:4 + kb + (t + 1) * TBL],
                    p_ps[64:128, :])
        if j > 0:  # chunk-boundary columns from X_prev's last 4 steps
            ka, kb = 2 * j - 1, 2 * j
            tl_ps = psT.tile([128, 4], DT, tag="tail", name=f"tl_ps{j}")
            for k in range(8):
                nc.tensor.matmul(tl_ps[:], w1sb[k][:, j * 128:(j + 1) * 128],
                                 xpsb[k][:, TC - 4:TC],
                                 start=(k == 0), stop=(k == 7))
            nc.scalar.copy(pext[j][0:64, 4:4 + ka], tl_ps[0:64, 4 - ka:4])
            nc.scalar.copy(pext[j][64:128, 4:4 + kb], tl_ps[64:128, 4 - kb:4])

    emit_p(0)  # j0 first: rows 0:64 are Bu1_own, needed by the own-scan chain

    # ---- Bu_own + own scans (time-major so CH1/Y of block 0 can start early)
    hsb = [hp.tile([128, TC], DT, tag=f"h{n}", name=f"h{n}") for n in range(8)]
    for t in range(NT):
        for n in range(8):
            bu_ps = psA.tile([128, TBL], DT, tag="mm", name=f"buo{n}_{t}")
            nc.tensor.matmul(bu_ps[:], b2sb[:, n * 128:(n + 1) * 128],
                             pext[0][0:64, 4 + t * TBL:4 + (t + 1) * TBL],
                             start=True, stop=True)
            init = iown[n][:] if t == 0 else hsb[n][:, TBL - 1:TBL]
            nc.vector.tensor_tensor_scan(
                hsb[n][:, t * TBL:(t + 1) * TBL],
                absb[n][:, t * TBL:(t + 1) * TBL], bu_ps[:], init,
                MULT, ADD)
        if t == 0:
            emit_p(1)  # PE work to overlap with the own scans
            emit_p(2)

    # ---- CH1 = (H @ C1)^T -> pext[0][0:64] (Bu1 is dead there by now) -------
    # ---- then Y^T = sum_m W2[m].T @ pext[m], per time block -----------------
    for t in range(NT):
        base = 4 + t * TBL
        c_ps = psB.tile([64, TBL], DT, tag="sm", name=f"c_ps{t}")
        for n in range(8):
            nc.tensor.matmul(c_ps[:], c1sb[n][:],
                             hsb[n][:, t * TBL:(t + 1) * TBL],
                             start=(n == 0), stop=(n == 7))
        nc.scalar.copy(pext[0][0:64, base:base + TBL], c_ps[:])

        for o in range(8):
            ob = slice(o * 128, (o + 1) * 128)
            y_ps = psA.tile([128, TBL], DT, tag="mm", name=f"y_ps{o}_{t}")
            for m in range(3):
                nc.tensor.matmul(y_ps[:], w2sb[m][:, ob],
                                 pext[m][:, base:base + TBL],
                                 start=(m == 0), stop=(m == 2))
            ysb = yp.tile([128, TBL], DT, tag="y", name=f"y{o}_{t}")
            nc.scalar.copy(ysb[:], y_ps[:])
            nc.sync.dma_start(yt[ob, t * TBL:(t + 1) * TBL], ysb[:])


def _build():
    nc = bacc.Bacc("TRN2", target_bir_lowering=False, debug=False,
                   num_devices=8)
    xo = nc.dram_tensor("xo", [D, TC], DT, kind="ExternalInput").ap()
    xp = nc.dram_tensor("xp", [D, TC], DT, kind="ExternalInput").ap()
    w1 = nc.dram_tensor("w1", [D, 64 + R * KX], DT, kind="ExternalInput").ap()
    b2 = nc.dram_tensor("b2", [R, NST], DT, kind="ExternalInput").ap()
    c1 = nc.dram_tensor("c1", [NST, R], DT, kind="ExternalInput").ap()
    w2 = nc.dram_tensor("w2", [64 + R * KX, OUT], DT, kind="ExternalInput").ap()
    av = nc.dram_tensor("av", [NST, 1], DT, kind="ExternalInput").ap()
    iv = nc.dram_tensor("iv", [NST, 1], DT, kind="ExternalInput").ap()
    ioff = nc.dram_tensor("ioff", [NST, 1], DT, kind="ExternalInput").ap()
    yt = nc.dram_tensor("yt", [OUT, TC], DT, kind="ExternalOutput").ap()

    with tile.TileContext(nc) as tc, ExitStack() as ctx:
        _emit(ctx, tc, (xo, xp, w1, b2, c1, w2, av, iv, ioff, yt))
    nc.compile()
    return nc


def _get_nc():
    global _CACHED_NC
    if _CACHED_NC is None:
        _CACHED_NC = _build()
    return _CACHED_NC


def kernel(inputs, h0, A, B1, B2, C1, C2, M1, M2):
    global LAST_RESULT
    X = np.ascontiguousarray(np.asarray(inputs, dtype=F32))
    h0 = np.asarray(h0, dtype=F32)
    A = np.asarray(A, dtype=F32)
    W1 = np.concatenate(
        [np.asarray(B1, dtype=F32)]
        + [np.ascontiguousarray(np.asarray(M1, dtype=F32)[:, :, k].T)
           for k in range(KX)], axis=1)
    W2 = np.concatenate(
        [np.asarray(C2, dtype=F32)]
        + [np.ascontiguousarray(np.asarray(M2, dtype=F32)[:, :, k].T)
           for k in range(KX)], axis=0)
    W1 = np.ascontiguousarray(W1)
    W2 = np.ascontiguousarray(W2)
    b2c = np.ascontiguousarray(np.asarray(B2, dtype=F32))
    c1c = np.ascontiguousarray(np.asarray(C1, dtype=F32))
    avc = np.ascontiguousarray(A.reshape(-1, 1))
    z = np.zeros((NST, 1), F32)
    h0c = np.ascontiguousarray(h0.reshape(-1, 1))

    in_maps = []
    for c in range(8):
        b, half = divmod(c, 2)
        xoc = np.ascontiguousarray(X[b, half * TC:(half + 1) * TC, :].T)
        if half == 0:
            xpc = np.zeros((D, TC), F32)
            ivc, ioc = z, h0c
        else:
            xpc = np.ascontiguousarray(X[b, 0:TC, :].T)
            ivc, ioc = h0c, z
        in_maps.append({"xo": xoc, "xp": xpc, "w1": W1, "b2": b2c,
                        "c1": c1c, "w2": W2, "av": avc, "iv": ivc,
                        "ioff": ioc})

    nc = _get_nc()
    trace = bool(int(os.environ.get("KERNEL_TRACE", "0")))
    LAST_RESULT = run_bass_kernel_spmd(nc, in_maps, core_ids=list(range(8)),
                                       trace=trace)
    Y = np.empty((B, T, OUT), F32)
    for c in range(8):
        b, half = divmod(c, 2)
        Y[b, half * TC:(half + 1) * TC, :] = LAST_RESULT.results[c]["yt"].T
    return Y


# revision 5
# speedup vs baseline: 1.5222x; 1.5222x over previous
"""Trainium2 Bass kernel for nn_LDS_LR: low-rank LDS + AR low-rank correction.

Math (per batch b):
    Bu   = X @ B1 @ B2                      # [T, N] rank-64 input projection
    h_t  = A * h_{t-1} + Bu_t               # diagonal recurrence, h_{-1} = h0
    lds  = H @ C1 @ C2                      # [T, O] rank-64 output projection
    proj = einsum('ti,rik->trk', X, M1)     # [T, R, KX]
    ar_t = sum_k M2[:,:,k] @ proj[t-k,:,k]  # AR with KX=5 taps
    Y    = lds + ar

Sharding: 8 cores = 4 batches x 2 sequence halves (1024 steps each).
Chunk-1 cores recompute the carry h_1023 on-device from the previous
half's inputs (rank-64 projection + a DVE carry scan), so the program is
fully uniform SPMD with no cross-core communication.

Device layout is transposed throughout: [feature, time]. Weights are
fused host-side:
    W1 [1024, 384] = [B1 | M1k0.T | ... | M1k4.T]   (input proj, one matmul)
    W2 [384, 1024] = [C2 ; M2k0.T ; ... ; M2k4.T]   (output proj, one matmul)
The AR delay taps become column shifts in the access patterns of the
final matmul's moving operand (P_ext keeps 4 leading boundary columns),
so no shift copies are ever materialized.
"""

import contextlib
import ctypes
import os
import sys
import types

import numpy as np
from contextlib import ExitStack

import concourse.bass as bass
import concourse.tile as tile
from concourse import bacc, mybir
from concourse.bass_utils import run_bass_kernel_spmd


def _install_ntff_hook():
    """Provide antenv.axon_hooks.get_axon_ntff_profile_hook if the image
    lacks it, driving NTFF capture via the libaxon_pjrt C ABI directly."""
    try:
        from antenv.axon_hooks import get_axon_ntff_profile_hook  # noqa: F401
        return
    except ImportError:
        pass
    so_path = "/opt/axon/libaxon_pjrt.so"
    hook = None
    if os.path.exists(so_path):
        lib = ctypes.CDLL(so_path)
        if hasattr(lib, "axon_start_nrt_profile"):
            lib.axon_start_nrt_profile.argtypes = [
                ctypes.POINTER(ctypes.c_int64), ctypes.c_size_t]
            lib.axon_start_nrt_profile.restype = ctypes.c_int64
            lib.axon_stop_nrt_profile.argtypes = [ctypes.c_char_p]
            lib.axon_stop_nrt_profile.restype = ctypes.c_int64

            @contextlib.contextmanager
            def _hook(output_dir, device_ids):
                import jax
                jax.devices()
                if device_ids:
                    ids = (ctypes.c_int64 * len(device_ids))(*device_ids)
                    rc = lib.axon_start_nrt_profile(ids, len(device_ids))
                else:
                    rc = lib.axon_start_nrt_profile(None, 0)
                if rc != 0:
                    raise RuntimeError(f"axon_start_nrt_profile rc={rc}")
                try:
                    yield
                finally:
                    n = lib.axon_stop_nrt_profile(str(output_dir).encode())
                    print(f"ntff profile: {n} file(s) -> {output_dir}",
                          file=sys.stderr)

            hook = _hook
    mod = types.ModuleType("antenv.axon_hooks")
    mod.get_axon_ntff_profile_hook = lambda: hook
    mod.set_axon_ntff_profile_hook = lambda h: None
    sys.modules["antenv.axon_hooks"] = mod


_install_ntff_hook()

DT = mybir.dt.float32
_MDT_NAME = os.environ.get("KERNEL_MDT", "bf16")
MDT = {"f32": mybir.dt.float32, "f32r": mybir.dt.float32r,
       "bf16": mybir.dt.bfloat16}[_MDT_NAME]
MNP = mybir.dt.np(MDT)
F32 = np.float32

B, T, D = 4, 2048, 1024
NST, R, KX, OUT = 1024, 64, 5, 1024
TC = 1024          # per-core chunk length
TBL = 512          # time block (one PSUM bank at fp32)
NT = TC // TBL     # 2 time blocks

_CACHED_NC = None
LAST_RESULT = None  # BassKernelResults of the most recent run (for test.py)

MULT = mybir.AluOpType.mult
ADD = mybir.AluOpType.add


def _emit(ctx, tc, io):
    nc = tc.nc
    xo, xp, w1, b2, c1, w2, av, iv, ioff, yt = io

    wp = ctx.enter_context(tc.tile_pool(name="wp", bufs=1))
    xpool = ctx.enter_context(tc.tile_pool(name="xpool", bufs=1))
    hp = ctx.enter_context(tc.tile_pool(name="hp", bufs=1))
    pp = ctx.enter_context(tc.tile_pool(name="pp", bufs=1))
    sc = ctx.enter_context(tc.tile_pool(name="sc", bufs=2))
    yp = ctx.enter_context(tc.tile_pool(name="yp", bufs=3))
    psA = ctx.enter_context(tc.tile_pool(name="psA", bufs=4, space="PSUM"))
    psB = ctx.enter_context(tc.tile_pool(name="psB", bufs=2, space="PSUM"))
    psT = ctx.enter_context(tc.tile_pool(name="psT", bufs=2, space="PSUM"))

    # ---- param / input loads -------------------------------------------------
    w1sb = []
    for k in range(8):
        t = wp.tile([128, 384], MDT, tag=f"w1_{k}", name=f"w1_{k}")
        nc.sync.dma_start(t[:], w1[k * 128:(k + 1) * 128, :])
        w1sb.append(t)
    b2sb = wp.tile([64, 1024], MDT, tag="b2", name="b2sb")
    nc.sync.dma_start(b2sb[:], b2[:])
    c1sb = []
    for k in range(8):
        t = wp.tile([128, 64], MDT, tag=f"c1_{k}", name=f"c1_{k}")
        nc.sync.dma_start(t[:], c1[k * 128:(k + 1) * 128, :])
        c1sb.append(t)
    w2sb = []
    for m in range(3):
        t = wp.tile([128, 1024], MDT, tag=f"w2_{m}", name=f"w2_{m}")
        nc.sync.dma_start(t[:], w2[m * 128:(m + 1) * 128, :])
        w2sb.append(t)
    avsb, ivsb, iosb = [], [], []
    for n in range(8):
        a = wp.tile([128, 1], DT, tag=f"av{n}", name=f"av{n}")
        nc.sync.dma_start(a[:], av[n * 128:(n + 1) * 128, :])
        avsb.append(a)
        v = wp.tile([128, 1], DT, tag=f"iv{n}", name=f"iv{n}")
        nc.sync.dma_start(v[:], iv[n * 128:(n + 1) * 128, :])
        ivsb.append(v)
        o = wp.tile([128, 1], DT, tag=f"io{n}", name=f"io{n}")
        nc.sync.dma_start(o[:], ioff[n * 128:(n + 1) * 128, :])
        iosb.append(o)
    xosb, xpsb = [], []
    for k in range(8):
        t = xpool.tile([128, 1024], MDT, tag=f"xo{k}", name=f"xo{k}")
        nc.sync.dma_start(t[:], xo[k * 128:(k + 1) * 128, :])
        xosb.append(t)
        t = xpool.tile([128, 1024], MDT, tag=f"xp{k}", name=f"xp{k}")
        nc.sync.dma_start(t[:], xp[k * 128:(k + 1) * 128, :])
        xpsb.append(t)

    # A broadcast along time: ab[n] = A[n-tile] * ones  (ScalarE, keeps DVE free)
    ones = wp.tile([128, 1024], DT, tag="ones", name="ones")
    nc.vector.memset(ones[:], 1.0)
    absb = []
    for n in range(8):
        ab = wp.tile([128, 1024], DT, tag=f"ab{n}", name=f"ab{n}")
        nc.scalar.mul(ab[:], ones[:], avsb[n][:])
        absb.append(ab)

    # ---- G_prev = (X_prev @ B1)^T : [64, 1024] ------------------------------
    gsb = wp.tile([64, 1024], MDT, tag="gprev", name="gsb")
    for t in range(NT):
        g_ps = psB.tile([64, TBL], DT, tag="sm", name=f"g_ps{t}")
        for k in range(8):
            nc.tensor.matmul(g_ps[:], w1sb[k][:, 0:64],
                             xpsb[k][:, t * TBL:(t + 1) * TBL],
                             start=(k == 0), stop=(k == 7))
        nc.scalar.copy(gsb[:, t * TBL:(t + 1) * TBL], g_ps[:])

    # ---- carry: Bu_prev per n-tile + DVE carry scan -> initial_own ----------
    iown = []
    for n in range(8):
        prev_col = None
        for t in range(NT):
            bu_ps = psA.tile([128, TBL], DT, tag="mm", name=f"bup{n}_{t}")
            nc.tensor.matmul(bu_ps[:], b2sb[:, n * 128:(n + 1) * 128],
                             gsb[0:64, t * TBL:(t + 1) * TBL],
                             start=True, stop=True)
            cs = sc.tile([128, TBL], DT, tag="cscan", name=f"cs{n}_{t}")
            init = ivsb[n][:] if t == 0 else prev_col
            nc.vector.tensor_tensor_scan(
                cs[:], absb[n][:, t * TBL:(t + 1) * TBL], bu_ps[:], init,
                MULT, ADD)
            prev_col = cs[:, TBL - 1:TBL]
        it = wp.tile([128, 1], DT, tag=f"iown{n}", name=f"iown{n}")
        nc.vector.tensor_scalar_add(it[:], prev_col, iosb[n][:])
        iown.append(it)

    # ---- P_ext = (X_own @ W1)^T, AR taps pre-shifted at write time ----------
    # pext[j] rows 0:64 / 64:128 hold taps (2j-1, 2j) for j>0; tap k's value
    # for time s is stored at column 4+s+k, so the final Y matmul reads all
    # 128 rows at ONE column range [4+t*TBL, ...) — full K=128, base 0, no
    # row-group pairing (which hangs fp32 HW).  j0 = [Bu1 | tap0], shift 0.
    PW = 4 + TC + 4
    pext = [pp.tile([128, PW], MDT, tag=f"pext{j}", name=f"pext{j}")
            for j in range(3)]

    def emit_p(j):
        for t in range(NT):
            p_ps = psA.tile([128, TBL], DT, tag="mm", name=f"p_ps{j}_{t}")
            for k in range(8):
                nc.tensor.matmul(p_ps[:], w1sb[k][:, j * 128:(j + 1) * 128],
                                 xosb[k][:, t * TBL:(t + 1) * TBL],
                                 start=(k == 0), stop=(k == 7))
            if j == 0:
                nc.scalar.copy(pext[0][:, 4 + t * TBL:4 + (t + 1) * TBL],
                               p_ps[:])
            else:
                ka, kb = 2 * j - 1, 2 * j
                nc.scalar.copy(
                    pext[j][0:64, 4 + ka + t * TBL:4 + ka + (t + 1) * TBL],
                    p_ps[0:64, :])
                nc.scalar.copy(
                    pext[j][64:128, 4 + kb + t * TBL:4 + kb + (t + 1) * TBL],
                    p_ps[64:128, :])
        if j > 0:  # chunk-boundary columns from X_prev's last 4 steps
            ka, kb = 2 * j - 1, 2 * j
            tl_ps = psT.tile([128, 4], DT, tag="tail", name=f"tl_ps{j}")
            for k in range(8):
                nc.tensor.matmul(tl_ps[:], w1sb[k][:, j * 128:(j + 1) * 128],
                                 xpsb[k][:, TC - 4:TC],
                                 start=(k == 0), stop=(k == 7))
            nc.scalar.copy(pext[j][0:64, 4:4 + ka], tl_ps[0:64, 4 - ka:4])
            nc.scalar.copy(pext[j][64:128, 4:4 + kb], tl_ps[64:128, 4 - kb:4])

    emit_p(0)  # j0 first: rows 0:64 are Bu1_own, needed by the own-scan chain

    # ---- Bu_own + own scans (time-major so CH1/Y of block 0 can start early)
    hsb = [hp.tile([128, TC], MDT, tag=f"h{n}", name=f"h{n}") for n in range(8)]
    for t in range(NT):
        for n in range(8):
            bu_ps = psA.tile([128, TBL], DT, tag="mm", name=f"buo{n}_{t}")
            nc.tensor.matmul(bu_ps[:], b2sb[:, n * 128:(n + 1) * 128],
                             pext[0][0:64, 4 + t * TBL:4 + (t + 1) * TBL],
                             start=True, stop=True)
            init = iown[n][:] if t == 0 else hsb[n][:, TBL - 1:TBL]
            nc.vector.tensor_tensor_scan(
                hsb[n][:, t * TBL:(t + 1) * TBL],
                absb[n][:, t * TBL:(t + 1) * TBL], bu_ps[:], init,
                MULT, ADD)
        if t == 0:
            emit_p(1)  # PE work to overlap with the own scans
            emit_p(2)

    # ---- CH1 = (H @ C1)^T -> pext[0][0:64] (Bu1 is dead there by now) -------
    # ---- then Y^T = sum_m W2[m].T @ pext[m], per time block -----------------
    for t in range(NT):
        base = 4 + t * TBL
        c_ps = psB.tile([64, TBL], DT, tag="sm", name=f"c_ps{t}")
        for n in range(8):
            nc.tensor.matmul(c_ps[:], c1sb[n][:],
                             hsb[n][:, t * TBL:(t + 1) * TBL],
                             start=(n == 0), stop=(n == 7))
        nc.scalar.copy(pext[0][0:64, base:base + TBL], c_ps[:])

        for o in range(8):
            ob = slice(o * 128, (o + 1) * 128)
            y_ps = psA.tile([128, TBL], DT, tag="mm", name=f"y_ps{o}_{t}")
            for m in range(3):
                nc.tensor.matmul(y_ps[:], w2sb[m][:, ob],
                                 pext[m][:, base:base + TBL],
                                 start=(m == 0), stop=(m == 2))
            ysb = yp.tile([128, TBL], DT, tag="y", name=f"y{o}_{t}")
            nc.scalar.copy(ysb[:], y_ps[:])
            nc.sync.dma_start(yt[ob, t * TBL:(t + 1) * TBL], ysb[:])


def _build():
    nc = bacc.Bacc("TRN2", target_bir_lowering=False, debug=False,
                   num_devices=8)
    xo = nc.dram_tensor("xo", [D, TC], MDT, kind="ExternalInput").ap()
    xp = nc.dram_tensor("xp", [D, TC], MDT, kind="ExternalInput").ap()
    w1 = nc.dram_tensor("w1", [D, 64 + R * KX], MDT, kind="ExternalInput").ap()
    b2 = nc.dram_tensor("b2", [R, NST], MDT, kind="ExternalInput").ap()
    c1 = nc.dram_tensor("c1", [NST, R], MDT, kind="ExternalInput").ap()
    w2 = nc.dram_tensor("w2", [64 + R * KX, OUT], MDT, kind="ExternalInput").ap()
    av = nc.dram_tensor("av", [NST, 1], DT, kind="ExternalInput").ap()
    iv = nc.dram_tensor("iv", [NST, 1], DT, kind="ExternalInput").ap()
    ioff = nc.dram_tensor("ioff", [NST, 1], DT, kind="ExternalInput").ap()
    yt = nc.dram_tensor("yt", [OUT, TC], DT, kind="ExternalOutput").ap()

    with tile.TileContext(nc) as tc, ExitStack() as ctx:
        _emit(ctx, tc, (xo, xp, w1, b2, c1, w2, av, iv, ioff, yt))
    nc.compile()
    return nc


def _get_nc():
    global _CACHED_NC
    if _CACHED_NC is None:
        _CACHED_NC = _build()
    return _CACHED_NC


def kernel(inputs, h0, A, B1, B2, C1, C2, M1, M2):
    global LAST_RESULT
    X = np.ascontiguousarray(np.asarray(inputs, dtype=F32))
    h0 = np.asarray(h0, dtype=F32)
    A = np.asarray(A, dtype=F32)
    W1 = np.concatenate(
        [np.asarray(B1, dtype=F32)]
        + [np.ascontiguousarray(np.asarray(M1, dtype=F32)[:, :, k].T)
           for k in range(KX)], axis=1)
    W2 = np.concatenate(
        [np.asarray(C2, dtype=F32)]
        + [np.ascontiguousarray(np.asarray(M2, dtype=F32)[:, :, k].T)
           for k in range(KX)], axis=0)
    W1 = np.ascontiguousarray(W1.astype(MNP))
    W2 = np.ascontiguousarray(W2.astype(MNP))
    b2c = np.ascontiguousarray(np.asarray(B2, dtype=F32).astype(MNP))
    c1c = np.ascontiguousarray(np.asarray(C1, dtype=F32).astype(MNP))
    avc = np.ascontiguousarray(A.reshape(-1, 1))
    z = np.zeros((NST, 1), F32)
    h0c = np.ascontiguousarray(h0.reshape(-1, 1))

    in_maps = []
    for c in range(8):
        b, half = divmod(c, 2)
        xoc = np.ascontiguousarray(X[b, half * TC:(half + 1) * TC, :].T.astype(MNP))
        if half == 0:
            xpc = np.zeros((D, TC), MNP)
            ivc, ioc = z, h0c
        else:
            xpc = np.ascontiguousarray(X[b, 0:TC, :].T.astype(MNP))
            ivc, ioc = h0c, z
        in_maps.append({"xo": xoc, "xp": xpc, "w1": W1, "b2": b2c,
                        "c1": c1c, "w2": W2, "av": avc, "iv": ivc,
                        "ioff": ioc})

    nc = _get_nc()
    trace = bool(int(os.environ.get("KERNEL_TRACE", "0")))
    LAST_RESULT = run_bass_kernel_spmd(nc, in_maps, core_ids=list(range(8)),
                                       trace=trace)
    Y = np.empty((B, T, OUT), F32)
    for c in range(8):
        b, half = divmod(c, 2)
        Y[b, half * TC:(half + 1) * TC, :] = LAST_RESULT.results[c]["yt"].T
    return Y


# revision 6
# speedup vs baseline: 1.9616x; 1.2887x over previous
"""Trainium2 Bass kernel for nn_LDS_LR: low-rank LDS + AR low-rank correction.

Math (per batch b):
    Bu   = X @ B1 @ B2                      # [T, N] rank-64 input projection
    h_t  = A * h_{t-1} + Bu_t               # diagonal recurrence, h_{-1} = h0
    lds  = H @ C1 @ C2                      # [T, O] rank-64 output projection
    proj = einsum('ti,rik->trk', X, M1)     # [T, R, KX]
    ar_t = sum_k M2[:,:,k] @ proj[t-k,:,k]  # AR with KX=5 taps
    Y    = lds + ar

Sharding: 8 cores = 4 batches x 2 sequence halves (1024 steps each).
Chunk-1 cores recompute the carry h_1023 on-device from the previous
half's inputs (rank-64 projection + a DVE carry scan), so the program is
fully uniform SPMD with no cross-core communication.

Device layout is transposed throughout: [feature, time]. Weights are
fused host-side:
    W1 [1024, 384] = [B1 | M1k0.T | ... | M1k4.T]   (input proj, one matmul)
    W2 [384, 1024] = [C2 ; M2k0.T ; ... ; M2k4.T]   (output proj, one matmul)
The AR delay taps become column shifts in the access patterns of the
final matmul's moving operand (P_ext keeps 4 leading boundary columns),
so no shift copies are ever materialized.
"""

import contextlib
import ctypes
import os
import sys
import types

import numpy as np
from contextlib import ExitStack

import concourse.bass as bass
import concourse.tile as tile
from concourse import bacc, mybir
from concourse.bass_utils import run_bass_kernel_spmd


def _install_ntff_hook():
    """Provide antenv.axon_hooks.get_axon_ntff_profile_hook if the image
    lacks it, driving NTFF capture via the libaxon_pjrt C ABI directly."""
    try:
        from antenv.axon_hooks import get_axon_ntff_profile_hook  # noqa: F401
        return
    except ImportError:
        pass
    so_path = "/opt/axon/libaxon_pjrt.so"
    hook = None
    if os.path.exists(so_path):
        lib = ctypes.CDLL(so_path)
        if hasattr(lib, "axon_start_nrt_profile"):
            lib.axon_start_nrt_profile.argtypes = [
                ctypes.POINTER(ctypes.c_int64), ctypes.c_size_t]
            lib.axon_start_nrt_profile.restype = ctypes.c_int64
            lib.axon_stop_nrt_profile.argtypes = [ctypes.c_char_p]
            lib.axon_stop_nrt_profile.restype = ctypes.c_int64

            @contextlib.contextmanager
            def _hook(output_dir, device_ids):
                import jax
                jax.devices()
                if device_ids:
                    ids = (ctypes.c_int64 * len(device_ids))(*device_ids)
                    rc = lib.axon_start_nrt_profile(ids, len(device_ids))
                else:
                    rc = lib.axon_start_nrt_profile(None, 0)
                if rc != 0:
                    raise RuntimeError(f"axon_start_nrt_profile rc={rc}")
                try:
                    yield
                finally:
                    n = lib.axon_stop_nrt_profile(str(output_dir).encode())
                    print(f"ntff profile: {n} file(s) -> {output_dir}",
                          file=sys.stderr)

            hook = _hook
    mod = types.ModuleType("antenv.axon_hooks")
    mod.get_axon_ntff_profile_hook = lambda: hook
    mod.set_axon_ntff_profile_hook = lambda h: None
    sys.modules["antenv.axon_hooks"] = mod


_install_ntff_hook()

DT = mybir.dt.float32
_MDT_NAME = os.environ.get("KERNEL_MDT", "f32r")
MDT = {"f32": mybir.dt.float32, "f32r": mybir.dt.float32r,
       "bf16": mybir.dt.bfloat16}[_MDT_NAME]
MNP = mybir.dt.np(MDT)
F32 = np.float32

B, T, D = 4, 2048, 1024
NST, R, KX, OUT = 1024, 64, 5, 1024
TC = 1024          # per-core chunk length
TBL = 512          # time block (one PSUM bank at fp32)
NT = TC // TBL     # 2 time blocks

_CACHED_NC = None
LAST_RESULT = None  # BassKernelResults of the most recent run (for test.py)

MULT = mybir.AluOpType.mult
ADD = mybir.AluOpType.add


def _emit(ctx, tc, io):
    nc = tc.nc
    xo, xp, w1, b2, c1, w2, av, iv, ioff, yt = io

    wp = ctx.enter_context(tc.tile_pool(name="wp", bufs=1))
    xpool = ctx.enter_context(tc.tile_pool(name="xpool", bufs=1))
    hp = ctx.enter_context(tc.tile_pool(name="hp", bufs=1))
    pp = ctx.enter_context(tc.tile_pool(name="pp", bufs=1))
    sc = ctx.enter_context(tc.tile_pool(name="sc", bufs=2))
    yp = ctx.enter_context(tc.tile_pool(name="yp", bufs=3))
    psA = ctx.enter_context(tc.tile_pool(name="psA", bufs=2, space="PSUM"))
    psBu = ctx.enter_context(tc.tile_pool(name="psBu", bufs=2, space="PSUM"))
    psB = ctx.enter_context(tc.tile_pool(name="psB", bufs=1, space="PSUM"))
    psT = ctx.enter_context(tc.tile_pool(name="psT", bufs=1, space="PSUM"))

    # ---- param / input loads (two DMA queues; first-needed first) ----------
    # sync queue: xp -> b2 -> av/iv/io -> c1 ; gpsimd queue: w1 -> xo -> w2.
    # G_prev (the first matmuls) needs only xp+w1, so both queues front-load it.
    xpsb = []
    for k in range(8):
        t = xpool.tile([128, 1024], MDT, tag=f"xp{k}", name=f"xp{k}")
        nc.sync.dma_start(t[:], xp[k * 128:(k + 1) * 128, :])
        xpsb.append(t)
    w1sb = []
    for k in range(8):
        t = wp.tile([128, 384], MDT, tag=f"w1_{k}", name=f"w1_{k}")
        nc.gpsimd.dma_start(t[:], w1[k * 128:(k + 1) * 128, :])
        w1sb.append(t)
    b2sb = wp.tile([64, 1024], MDT, tag="b2", name="b2sb")
    nc.sync.dma_start(b2sb[:], b2[:])
    avsb, ivsb, iosb = [], [], []
    for n in range(8):
        a = wp.tile([128, 1], DT, tag=f"av{n}", name=f"av{n}")
        nc.sync.dma_start(a[:], av[n * 128:(n + 1) * 128, :])
        avsb.append(a)
        v = wp.tile([128, 1], DT, tag=f"iv{n}", name=f"iv{n}")
        nc.sync.dma_start(v[:], iv[n * 128:(n + 1) * 128, :])
        ivsb.append(v)
        o = wp.tile([128, 1], DT, tag=f"io{n}", name=f"io{n}")
        nc.sync.dma_start(o[:], ioff[n * 128:(n + 1) * 128, :])
        iosb.append(o)
    xosb = []
    for k in range(8):
        t = xpool.tile([128, 1024], MDT, tag=f"xo{k}", name=f"xo{k}")
        nc.gpsimd.dma_start(t[:], xo[k * 128:(k + 1) * 128, :])
        xosb.append(t)
    c1sb = []
    for k in range(8):
        t = wp.tile([128, 64], DT if False else MDT, tag=f"c1_{k}", name=f"c1_{k}")
        nc.sync.dma_start(t[:], c1[k * 128:(k + 1) * 128, :])
        c1sb.append(t)
    w2sb = []
    for m in range(3):
        t = wp.tile([128, 1024], MDT, tag=f"w2_{m}", name=f"w2_{m}")
        nc.gpsimd.dma_start(t[:], w2[m * 128:(m + 1) * 128, :])
        w2sb.append(t)

    # A broadcast along time: ab[n] = A[n-tile] * ones  (ScalarE, keeps DVE free)
    ones = wp.tile([128, 1024], DT, tag="ones", name="ones")
    nc.vector.memset(ones[:], 1.0)
    absb = []
    for n in range(8):
        ab = wp.tile([128, 1024], DT, tag=f"ab{n}", name=f"ab{n}")
        nc.scalar.mul(ab[:], ones[:], avsb[n][:])
        absb.append(ab)

    # ---- G_prev = (X_prev @ B1)^T : [64, 1024] ------------------------------
    gsb = wp.tile([64, 1024], MDT, tag="gprev", name="gsb")
    for t in range(NT):
        g_ps = psB.tile([64, TBL], DT, tag="sm", name=f"g_ps{t}")
        for k in range(8):
            nc.tensor.matmul(g_ps[:], w1sb[k][:, 0:64],
                             xpsb[k][:, t * TBL:(t + 1) * TBL],
                             start=(k == 0), stop=(k == 7))
        nc.scalar.copy(gsb[:, t * TBL:(t + 1) * TBL], g_ps[:])

    # ---- carry: Bu_prev per n-tile + DVE carry scan -> initial_own ----------
    iown = []
    for n in range(8):
        bu_ps = psBu.tile([128, TC], DT, tag="bu", name=f"bup{n}")
        for t in range(NT):
            nc.tensor.matmul(bu_ps[:, t * TBL:(t + 1) * TBL],
                             b2sb[:, n * 128:(n + 1) * 128],
                             gsb[0:64, t * TBL:(t + 1) * TBL],
                             start=True, stop=True)
        cs = sc.tile([128, TC], DT, tag="cscan", name=f"cs{n}")
        nc.vector.tensor_tensor_scan(cs[:], absb[n][:], bu_ps[:], ivsb[n][:],
                                     MULT, ADD)
        it = wp.tile([128, 1], DT, tag=f"iown{n}", name=f"iown{n}")
        nc.vector.tensor_scalar_add(it[:], cs[:, TC - 1:TC], iosb[n][:])
        iown.append(it)

    # ---- P_ext = (X_own @ W1)^T, AR taps pre-shifted at write time ----------
    # pext[j] rows 0:64 / 64:128 hold taps (2j-1, 2j) for j>0; tap k's value
    # for time s is stored at column 4+s+k, so the final Y matmul reads all
    # 128 rows at ONE column range [4+t*TBL, ...) — full K=128, base 0, no
    # row-group pairing (which hangs fp32 HW).  j0 = [Bu1 | tap0], shift 0.
    PW = 4 + TC + 4
    pext = [pp.tile([128, PW], MDT, tag=f"pext{j}", name=f"pext{j}")
            for j in range(3)]

    def emit_p(j):
        for t in range(NT):
            p_ps = psA.tile([128, TBL], DT, tag="mm", name=f"p_ps{j}_{t}")
            for k in range(8):
                nc.tensor.matmul(p_ps[:], w1sb[k][:, j * 128:(j + 1) * 128],
                                 xosb[k][:, t * TBL:(t + 1) * TBL],
                                 start=(k == 0), stop=(k == 7))
            if j == 0:
                nc.scalar.copy(pext[0][:, 4 + t * TBL:4 + (t + 1) * TBL],
                               p_ps[:])
            else:
                ka, kb = 2 * j - 1, 2 * j
                nc.scalar.copy(
                    pext[j][0:64, 4 + ka + t * TBL:4 + ka + (t + 1) * TBL],
                    p_ps[0:64, :])
                nc.scalar.copy(
                    pext[j][64:128, 4 + kb + t * TBL:4 + kb + (t + 1) * TBL],
                    p_ps[64:128, :])
        if j > 0:  # chunk-boundary columns from X_prev's last 4 steps
            ka, kb = 2 * j - 1, 2 * j
            tl_ps = psT.tile([128, 4], DT, tag="tail", name=f"tl_ps{j}")
            for k in range(8):
                nc.tensor.matmul(tl_ps[:], w1sb[k][:, j * 128:(j + 1) * 128],
                                 xpsb[k][:, TC - 4:TC],
                                 start=(k == 0), stop=(k == 7))
            nc.scalar.copy(pext[j][0:64, 4:4 + ka], tl_ps[0:64, 4 - ka:4])
            nc.scalar.copy(pext[j][64:128, 4:4 + kb], tl_ps[64:128, 4 - kb:4])

    emit_p(0)  # j0 first: rows 0:64 are Bu1_own, needed by the own-scan chain

    # ---- Bu_own + own scans (time-major so CH1/Y of block 0 can start early)
    hsb = [hp.tile([128, TC], MDT, tag=f"h{n}", name=f"h{n}") for n in range(8)]
    for n in range(8):
        bu_ps = psBu.tile([128, TC], DT, tag="bu", name=f"buo{n}")
        for t in range(NT):
            nc.tensor.matmul(bu_ps[:, t * TBL:(t + 1) * TBL],
                             b2sb[:, n * 128:(n + 1) * 128],
                             pext[0][0:64, 4 + t * TBL:4 + (t + 1) * TBL],
                             start=True, stop=True)
        nc.vector.tensor_tensor_scan(hsb[n][:], absb[n][:], bu_ps[:],
                                     iown[n][:], MULT, ADD)
        if n == 0:
            emit_p(1)  # PE work to overlap with the own scans
        elif n == 4:
            emit_p(2)

    # ---- CH1 = (H @ C1)^T -> pext[0][0:64] (Bu1 is dead there by now) -------
    # ---- then Y^T = sum_m W2[m].T @ pext[m], per time block -----------------
    for t in range(NT):
        base = 4 + t * TBL
        c_ps = psB.tile([64, TBL], DT, tag="sm", name=f"c_ps{t}")
        for n in range(8):
            nc.tensor.matmul(c_ps[:], c1sb[n][:],
                             hsb[n][:, t * TBL:(t + 1) * TBL],
                             start=(n == 0), stop=(n == 7))
        nc.scalar.copy(pext[0][0:64, base:base + TBL], c_ps[:])

        for o in range(8):
            ob = slice(o * 128, (o + 1) * 128)
            y_ps = psA.tile([128, TBL], DT, tag="mm", name=f"y_ps{o}_{t}")
            for m in range(3):
                nc.tensor.matmul(y_ps[:], w2sb[m][:, ob],
                                 pext[m][:, base:base + TBL],
                                 start=(m == 0), stop=(m == 2))
            ysb = yp.tile([128, TBL], DT, tag="y", name=f"y{o}_{t}")
            nc.scalar.copy(ysb[:], y_ps[:])
            nc.sync.dma_start(yt[ob, t * TBL:(t + 1) * TBL], ysb[:])


def _build():
    nc = bacc.Bacc("TRN2", target_bir_lowering=False, debug=False,
                   num_devices=8)
    xo = nc.dram_tensor("xo", [D, TC], MDT, kind="ExternalInput").ap()
    xp = nc.dram_tensor("xp", [D, TC], MDT, kind="ExternalInput").ap()
    w1 = nc.dram_tensor("w1", [D, 64 + R * KX], MDT, kind="ExternalInput").ap()
    b2 = nc.dram_tensor("b2", [R, NST], MDT, kind="ExternalInput").ap()
    c1 = nc.dram_tensor("c1", [NST, R], MDT, kind="ExternalInput").ap()
    w2 = nc.dram_tensor("w2", [64 + R * KX, OUT], MDT, kind="ExternalInput").ap()
    av = nc.dram_tensor("av", [NST, 1], DT, kind="ExternalInput").ap()
    iv = nc.dram_tensor("iv", [NST, 1], DT, kind="ExternalInput").ap()
    ioff = nc.dram_tensor("ioff", [NST, 1], DT, kind="ExternalInput").ap()
    yt = nc.dram_tensor("yt", [OUT, TC], DT, kind="ExternalOutput").ap()

    with tile.TileContext(nc) as tc, ExitStack() as ctx:
        _emit(ctx, tc, (xo, xp, w1, b2, c1, w2, av, iv, ioff, yt))
    nc.compile()
    return nc


def _get_nc():
    global _CACHED_NC
    if _CACHED_NC is None:
        _CACHED_NC = _build()
    return _CACHED_NC


def kernel(inputs, h0, A, B1, B2, C1, C2, M1, M2):
    global LAST_RESULT
    X = np.ascontiguousarray(np.asarray(inputs, dtype=F32))
    h0 = np.asarray(h0, dtype=F32)
    A = np.asarray(A, dtype=F32)
    W1 = np.concatenate(
        [np.asarray(B1, dtype=F32)]
        + [np.ascontiguousarray(np.asarray(M1, dtype=F32)[:, :, k].T)
           for k in range(KX)], axis=1)
    W2 = np.concatenate(
        [np.asarray(C2, dtype=F32)]
        + [np.ascontiguousarray(np.asarray(M2, dtype=F32)[:, :, k].T)
           for k in range(KX)], axis=0)
    W1 = np.ascontiguousarray(W1.astype(MNP))
    W2 = np.ascontiguousarray(W2.astype(MNP))
    b2c = np.ascontiguousarray(np.asarray(B2, dtype=F32).astype(MNP))
    c1c = np.ascontiguousarray(np.asarray(C1, dtype=F32).astype(MNP))
    avc = np.ascontiguousarray(A.reshape(-1, 1))
    z = np.zeros((NST, 1), F32)
    h0c = np.ascontiguousarray(h0.reshape(-1, 1))

    in_maps = []
    for c in range(8):
        b, half = divmod(c, 2)
        xoc = np.ascontiguousarray(X[b, half * TC:(half + 1) * TC, :].T.astype(MNP))
        if half == 0:
            xpc = np.zeros((D, TC), MNP)
            ivc, ioc = z, h0c
        else:
            xpc = np.ascontiguousarray(X[b, 0:TC, :].T.astype(MNP))
            ivc, ioc = h0c, z
        in_maps.append({"xo": xoc, "xp": xpc, "w1": W1, "b2": b2c,
                        "c1": c1c, "w2": W2, "av": avc, "iv": ivc,
                        "ioff": ioc})

    nc = _get_nc()
    trace = bool(int(os.environ.get("KERNEL_TRACE", "0")))
    LAST_RESULT = run_bass_kernel_spmd(nc, in_maps, core_ids=list(range(8)),
                                       trace=trace)
    Y = np.empty((B, T, OUT), F32)
    for c in range(8):
        b, half = divmod(c, 2)
        Y[b, half * TC:(half + 1) * TC, :] = LAST_RESULT.results[c]["yt"].T
    return Y
